# revision 6
# baseline (speedup 1.0000x reference)
"""GCNConv Bass kernel for Trainium2, 8 NeuronCores (axon).

Math (per reference):
    deg[n]  = in-degree of n over col (incl. self-loops)
    dis[n]  = rsqrt(deg[n])
    out     = D^-1/2 (A + I) D^-1/2 x W^T + b
Aggregate-first formulation:
    x2      = dis * x                        (row-scaled, fp16)
    agg[:, d] = sum_{e: col_e = d} x2[row_e]   (segment-sum via PE matmuls)
    out[d]  = dis[d] * (agg[:, d]^T @ W^T) + b

Sharding: destination nodes are split across 8 cores (1280 per core,
node range padded 10000 -> 10240); x / W / b replicated. Edges are
CSR-sorted by destination on host and padded so every 16-destination
group owns a whole number of 128-edge chunks, identical chunk->group
structure on every core (single SPMD program).

Device pipeline per core:
  1. deg -> dis via DVE reciprocal + ACT sqrt + 1 Newton step
  2. x2 = dis*x -> DRAM (fp16)
  3. dma_gather x2[row_e] in 8192-edge batches -> G tiles [128e x 128f]
  4. Sel[e, j] = (ld_e == j) via iota + is_equal (batched)
  5. PE: agg[:, group] += G^T @ Sel   (PSUM f32 accumulate)
  6. per 128-dest block: fin = agg_blk^T @ W^T, out = fin*dis_d + b
"""

import os
import sys
import types

import numpy as np

N_NODES = 10000
N_EDGES = 640000
C = 128
NCORES = 8
DPC = 1280              # dest nodes per core (padded)
N_PAD = DPC * NCORES    # 10240
GROUP = 16
NGRP = DPC // GROUP     # 80 groups per core
NT = N_PAD // 128       # 80 node tiles for deg/dis
NXT = (N_NODES + 127) // 128  # 79 x tiles (last has 16 rows)
NDB = DPC // 128        # 10 dest blocks per core
BATCH_CH = 64           # gather batch = 64 chunks = 8192 edges

_cache = {}
last_exec_time_ns = None
_STAGE = os.environ.get("KERNEL_STAGE", "full")  # x2 | gather | sel | agg | full


def _install_ntff_shim():
    if "antenv.axon_hooks" in sys.modules:
        return
    mod = types.ModuleType("antenv.axon_hooks")
    mod._hook = None
    mod.set_axon_ntff_profile_hook = lambda h: setattr(mod, "_hook", h)
    mod.get_axon_ntff_profile_hook = lambda: mod._hook
    sys.modules["antenv.axon_hooks"] = mod
    try:
        import antenv
        antenv.axon_hooks = mod
        from trn_agent_boot.trn_boot import _ntff_profile_via_ctypes
        mod._hook = _ntff_profile_via_ctypes("/opt/axon/libaxon_pjrt.so")
    except Exception:
        pass


def _wrap16(a):
    """[n] -> [128, n//16] int16, idx i at (i%16, i//16), replicated x8."""
    s = len(a) // 16
    w = a.reshape(s, 16).T
    return np.ascontiguousarray(np.tile(w, (8, 1)), dtype=np.int16)


def _prep(edge_index):
    row = edge_index[0].astype(np.int64)
    col = edge_index[1].astype(np.int64)
    loops = np.arange(N_NODES, dtype=np.int64)
    row = np.concatenate([row, loops])
    col = np.concatenate([col, loops])
    order = np.argsort(col, kind="stable")
    row = row[order]
    col = col[order]
    counts = np.bincount(col, minlength=N_PAD)
    rp = np.zeros(N_PAD + 1, dtype=np.int64)
    rp[1:] = np.cumsum(counts)

    # chunks per 16-dest group: max over cores, >= 1
    mch = np.ones(NGRP, dtype=np.int64)
    for c in range(NCORES):
        base = c * DPC
        segs = rp[base + GROUP : base + DPC + 1 : GROUP] - rp[base : base + DPC : GROUP]
        need = np.maximum(1, -(-segs // 128))
        mch = np.maximum(mch, need)
    nch_tot = int(mch.sum())
    grp_c0 = np.zeros(NGRP, dtype=np.int64)
    grp_c0[1:] = np.cumsum(mch)[:-1]

    epc = nch_tot * 128
    src_all = np.zeros((NCORES, epc), dtype=np.int64)
    ld_all = np.full((NCORES, epc), -1.0, dtype=np.float32)
    for c in range(NCORES):
        for g in range(NGRP):
            d0 = c * DPC + g * GROUP
            s, e = rp[d0], rp[d0 + GROUP]
            n = e - s
            o = grp_c0[g] * 128
            src_all[c, o : o + n] = row[s:e]
            ld_all[c, o : o + n] = col[s:e] - d0

    # rp wrapped column-major [128, NT] for device deg computation
    rpa = rp[:N_PAD].reshape(NT, 128).T.astype(np.int32)
    rpb = rp[1 : N_PAD + 1].reshape(NT, 128).T.astype(np.int32)

    idx_w = [_wrap16(src_all[c].astype(np.int16)) for c in range(NCORES)]
    ld_w = [
        np.ascontiguousarray(ld_all[c].reshape(nch_tot, 128).T, dtype=np.float32)
        for c in range(NCORES)
    ]
    return mch, nch_tot, idx_w, ld_w, rpa, rpb


def _build(mch, nch_tot):
    import concourse.bacc as bacc
    import concourse.tile as tile
    from concourse import mybir

    f32 = mybir.dt.float32
    f16 = mybir.dt.float16
    i32 = mybir.dt.int32
    i16 = mybir.dt.int16

    # chunk -> group map and group first/last chunk
    grp_of = np.repeat(np.arange(NGRP), mch)
    grp_c0 = np.zeros(NGRP, dtype=np.int64)
    grp_c0[1:] = np.cumsum(mch)[:-1]
    grp_last = grp_c0 + mch - 1

    batches = []
    b0 = 0
    while b0 < nch_tot:
        nb = min(BATCH_CH, nch_tot - b0)
        batches.append((b0, nb))
        b0 += nb

    nc = bacc.Bacc("TRN2", target_bir_lowering=False)
    x_in = nc.dram_tensor("x", [N_NODES, C], f32, kind="ExternalInput")
    wt_in = nc.dram_tensor("wt", [C, C], f32, kind="ExternalInput")   # W^T (inc, outc)
    b_in = nc.dram_tensor("b", [1, C], f32, kind="ExternalInput")
    rpa_in = nc.dram_tensor("rpa", [128, NT], i32, kind="ExternalInput")
    rpb_in = nc.dram_tensor("rpb", [128, NT], i32, kind="ExternalInput")
    rpao_in = nc.dram_tensor("rpao", [128, NDB], i32, kind="ExternalInput")
    rpbo_in = nc.dram_tensor("rpbo", [128, NDB], i32, kind="ExternalInput")
    idx_in = nc.dram_tensor("idx", [128, nch_tot * 8], i16, kind="ExternalInput")
    ld_in = nc.dram_tensor("ld", [128, nch_tot], f32, kind="ExternalInput")
    out_t = nc.dram_tensor("out", [DPC, C], f32, kind="ExternalOutput")

    with tile.TileContext(nc) as tc:
        with (
            tc.tile_pool(name="const", bufs=1) as cp,
            tc.tile_pool(name="xload", bufs=4) as xp,
            tc.tile_pool(name="x2w", bufs=4) as x2p,
            tc.tile_pool(name="dram", bufs=1, space="DRAM") as dp,
            tc.tile_pool(name="gath", bufs=2) as gp,
            tc.tile_pool(name="sel", bufs=2) as selp,
            tc.tile_pool(name="epi", bufs=2) as ep,
            tc.tile_pool(name="psum", bufs=1, space="PSUM") as pp,
            tc.tile_pool(name="psum2", bufs=2, space="PSUM") as pp2,
        ):
            # ---- constant loads ----
            idx_sb = cp.tile([128, nch_tot * 8], i16)
            nc.sync.dma_start(out=idx_sb[:], in_=idx_in[:])
            ld_sb = cp.tile([128, nch_tot], f32)
            nc.sync.dma_start(out=ld_sb[:], in_=ld_in[:])
            wt_sb = cp.tile([C, C], f32)
            nc.sync.dma_start(out=wt_sb[:], in_=wt_in[:])
            b_row = cp.tile([1, C], f32)
            nc.sync.dma_start(out=b_row[:], in_=b_in[:])
            rpa_sb = cp.tile([128, NT], i32)
            nc.sync.dma_start(out=rpa_sb[:], in_=rpa_in[:])
            rpb_sb = cp.tile([128, NT], i32)
            nc.sync.dma_start(out=rpb_sb[:], in_=rpb_in[:])
            rpao_sb = cp.tile([128, NDB], i32)
            nc.sync.dma_start(out=rpao_sb[:], in_=rpao_in[:])
            rpbo_sb = cp.tile([128, NDB], i32)
            nc.sync.dma_start(out=rpbo_sb[:], in_=rpbo_in[:])

            # iota j in 0..15 repeated BATCH_CH times
            iota_i = cp.tile([128, BATCH_CH * GROUP], i32)
            nc.gpsimd.iota(
                iota_i[:], pattern=[[0, BATCH_CH], [1, GROUP]], base=0,
                channel_multiplier=0,
            )
            iota_f = cp.tile([128, BATCH_CH * GROUP], f32)
            nc.vector.tensor_copy(out=iota_f[:], in_=iota_i[:])

            # b broadcast to all partitions: ones[1,128]^T @ b_row[1,128]
            ones1 = cp.tile([1, 128], f32)
            nc.vector.memset(ones1[:], 1.0)
            bbc_ps = pp2.tile([128, C], f32, space="PSUM", tag="bbc")
            nc.tensor.matmul(out=bbc_ps[:], lhsT=ones1[:], rhs=b_row[:],
                             start=True, stop=True)
            b_bc = cp.tile([128, C], f32)
            nc.vector.tensor_copy(out=b_bc[:], in_=bbc_ps[:])

            # ---- dis = rsqrt(max(deg,1)), deg = rpb - rpa ----
            def make_dis(rb, ra, ncols, tag):
                d_i = cp.tile([128, ncols], i32, tag=f"{tag}di")
                nc.vector.tensor_tensor(out=d_i[:], in0=rb[:], in1=ra[:],
                                        op=mybir.AluOpType.subtract)
                d_f = cp.tile([128, ncols], f32, tag=f"{tag}df")
                nc.vector.tensor_copy(out=d_f[:], in_=d_i[:])
                d_c = cp.tile([128, ncols], f32, tag=f"{tag}dc")
                nc.vector.tensor_scalar_max(d_c[:], d_f[:], 1.0)
                rec = cp.tile([128, ncols], f32, tag=f"{tag}rc")
                nc.vector.reciprocal(out=rec[:], in_=d_c[:])
                s0 = cp.tile([128, ncols], f32, tag=f"{tag}s0")
                nc.scalar.sqrt(s0[:], rec[:])
                # Newton: y = s*(1.5 - 0.5*deg*s^2)
                u = cp.tile([128, ncols], f32, tag=f"{tag}u")
                nc.vector.tensor_tensor(out=u[:], in0=s0[:], in1=s0[:],
                                        op=mybir.AluOpType.mult)
                nc.vector.tensor_tensor(out=u[:], in0=u[:], in1=d_c[:],
                                        op=mybir.AluOpType.mult)
                nc.vector.tensor_scalar(
                    out=u[:], in0=u[:], scalar1=-0.5, scalar2=1.5,
                    op0=mybir.AluOpType.mult, op1=mybir.AluOpType.add,
                )
                dis = cp.tile([128, ncols], f32, tag=f"{tag}dis")
                nc.vector.tensor_tensor(out=dis[:], in0=s0[:], in1=u[:],
                                        op=mybir.AluOpType.mult)
                return dis

            dis = make_dis(rpb_sb, rpa_sb, NT, "g")
            dis_own = make_dis(rpbo_sb, rpao_sb, NDB, "o")

            # ---- x2 = dis * x -> DRAM fp16 ----
            x2_dram = dp.tile([NXT * 128, C], f16)
            for t in range(NXT):
                h = min(128, N_NODES - t * 128)
                xt = xp.tile([128, C], f32, tag="xt")
                eng = nc.sync if t % 2 == 0 else nc.scalar
                eng.dma_start(out=xt[:h, :], in_=x_in[t * 128 : t * 128 + h, :])
                x2t = x2p.tile([128, C], f16, tag="x2t")
                if h < 128:
                    nc.vector.memset(x2t[:], 0.0)
                nc.vector.tensor_tensor(
                    out=x2t[:h, :], in0=xt[:h, :],
                    in1=dis[0:h, t : t + 1].to_broadcast([h, C]),
                    op=mybir.AluOpType.mult,
                )
                eng2 = nc.scalar if t % 2 == 0 else nc.sync
                nrows = 128 if h < 128 else h
                eng2.dma_start(
                    out=x2_dram[t * 128 : t * 128 + nrows, :], in_=x2t[:nrows, :]
                )

            # ---- gather + Sel + PE accumulate ----
            stage = _STAGE
            agg = pp.tile([128, DPC], f32, space="PSUM")
            if stage == "x2":
                xchk16 = xp.tile([128, C], f16, tag="xchk16")
                nc.sync.dma_start(out=xchk16[:], in_=x2_dram[0:128, :])
                xchk = xp.tile([128, C], f32, tag="xchk")
                nc.vector.tensor_copy(out=xchk[:], in_=xchk16[:])
                for bi in range(NDB):
                    nc.sync.dma_start(
                        out=out_t[bi * 128 : (bi + 1) * 128, :], in_=xchk[:]
                    )
            if stage in ("gather", "sel", "agg", "full"):
                for b0, nb in batches:
                    g_t = gp.tile([128, BATCH_CH * C], f16, tag="g")
                    nc.gpsimd.dma_gather(
                        out_ap=g_t[:, : nb * C].rearrange("p (k f) -> p k f", f=C),
                        in_ap=x2_dram[:, :],
                        idxs_ap=idx_sb[:, b0 * 8 : (b0 + nb) * 8],
                        num_idxs=nb * 128,
                        num_idxs_reg=nb * 128,
                        elem_size=C,
                        single_packet=False,
                    )
                    if stage in ("sel", "agg", "full"):
                        sel_t = selp.tile([128, BATCH_CH * GROUP], f16, tag="sel")
                        nc.vector.tensor_tensor(
                            out=sel_t[:, : nb * GROUP].rearrange(
                                "p (k j) -> p k j", j=GROUP
                            ),
                            in0=iota_f[:, : nb * GROUP].rearrange(
                                "p (k j) -> p k j", j=GROUP
                            ),
                            in1=ld_sb[:, b0 : b0 + nb].to_broadcast([128, nb, GROUP]),
                            op=mybir.AluOpType.is_equal,
                        )
                    if stage in ("agg", "full"):
                        for k in range(nb):
                            ch = b0 + k
                            g = int(grp_of[ch])
                            nc.tensor.matmul(
                                out=agg[:, g * GROUP : (g + 1) * GROUP],
                                lhsT=g_t[:, k * C : (k + 1) * C],
                                rhs=sel_t[:, k * GROUP : (k + 1) * GROUP],
                                start=(ch == int(grp_c0[g])),
                                stop=(ch == int(grp_last[g])),
                            )
                    else:
                        gc = gp.tile([128, C], f32, tag="gchk")
                        nc.vector.tensor_copy(out=gc[:], in_=g_t[:, :C])
                if stage in ("gather", "sel"):
                    for bi in range(NDB):
                        zz = ep.tile([128, 128], f32, tag="zz")
                        nc.vector.memset(zz[:], 0.0)
                        nc.sync.dma_start(
                            out=out_t[bi * 128 : (bi + 1) * 128, :], in_=zz[:]
                        )

            # ---- epilogue: project, scale, bias, store ----
            if stage in ("agg", "full"):
                for bi in range(NDB):
                    agg_sb = ep.tile([128, 128], f32, tag="aggs")
                    nc.vector.tensor_copy(
                        out=agg_sb[:], in_=agg[:, bi * 128 : (bi + 1) * 128]
                    )
                    if stage == "agg":
                        nc.sync.dma_start(
                            out=out_t[bi * 128 : (bi + 1) * 128, :], in_=agg_sb[:]
                        )
                        continue
                    fin = pp2.tile([128, 128], f32, space="PSUM", tag="fin")
                    nc.tensor.matmul(out=fin[:], lhsT=agg_sb[:], rhs=wt_sb[:],
                                     start=True, stop=True)
                    t1 = ep.tile([128, 128], f32, tag="t1")
                    nc.vector.tensor_tensor(
                        out=t1[:], in0=fin[:],
                        in1=dis_own[:, bi : bi + 1].to_broadcast([128, 128]),
                        op=mybir.AluOpType.mult,
                    )
                    t2 = ep.tile([128, 128], f32, tag="t2")
                    nc.vector.tensor_tensor(out=t2[:], in0=t1[:], in1=b_bc[:],
                                            op=mybir.AluOpType.add)
                    eng = nc.sync if bi % 2 == 0 else nc.scalar
                    eng.dma_start(out=out_t[bi * 128 : (bi + 1) * 128, :], in_=t2[:])
    nc.finalize()
    return nc


def kernel(x, edge_index, W, b):
    global last_exec_time_ns
    from concourse.bass_utils import run_bass_kernel_spmd

    x = np.ascontiguousarray(x, dtype=np.float32)
    edge_index = np.ascontiguousarray(edge_index, dtype=np.int32)
    W = np.ascontiguousarray(W, dtype=np.float32)
    b = np.ascontiguousarray(b, dtype=np.float32)

    mch, nch_tot, idx_w, ld_w, rpa, rpb = _prep(edge_index)

    key = (nch_tot, tuple(mch.tolist()))
    if key not in _cache:
        _cache.clear()
        _cache[key] = _build(mch, nch_tot)
    nc = _cache[key]

    wt = np.ascontiguousarray(W.T)
    b_row = b.reshape(1, C)
    in_maps = []
    for c in range(NCORES):
        in_maps.append({
            "x": x,
            "wt": wt,
            "b": b_row,
            "rpa": rpa,
            "rpb": rpb,
            "rpao": np.ascontiguousarray(rpa[:, c * NDB : (c + 1) * NDB]),
            "rpbo": np.ascontiguousarray(rpb[:, c * NDB : (c + 1) * NDB]),
            "idx": idx_w[c],
            "ld": ld_w[c],
        })

    trace = os.environ.get("KERNEL_TRACE", "0") == "1"
    if trace:
        _install_ntff_shim()
    r = run_bass_kernel_spmd(
        nc, in_maps, core_ids=list(range(NCORES)), trace=trace,
        trace_cores=list(range(NCORES)) if trace else None,
    )
    last_exec_time_ns = r.exec_time_ns
    out = np.concatenate([r.results[c]["out"] for c in range(NCORES)], axis=0)
    return np.ascontiguousarray(out[:N_NODES])


if __name__ == "__main__":
    rng = np.random.default_rng(0)
    x = rng.standard_normal((N_NODES, C)).astype(np.float32)
    ei = rng.integers(0, N_NODES, (2, N_EDGES)).astype(np.int32)
    W = rng.standard_normal((C, C)).astype(np.float32) * 0.1
    b = np.zeros(C, dtype=np.float32)
    out = kernel(x, ei, W, b)
    print("out", out.shape, out.dtype, float(np.abs(out).max()))
